# revision 8
# baseline (speedup 1.0000x reference)
"""Trainium2 Bass kernel for nn_CaptioningRNN (attention LSTM over T=64 steps).

Strategy (8-way tensor-parallel over H / gate dim):
 - Core k owns h-slice hk = [128k, 128k+128) and gate columns
   cols_k = {1024*g + 128k .. +128 : g in i,f,o,g}.
 - A is sharded by h for scores (A1: [h_loc, n, l]) and by l for the
   attention readout (A2: [l, n, h_loc]); both bf16, resident in SBUF.
 - Per-sample attention einsums run on the PE as block-diagonal matmuls
   with zero-padded lhsT tiles; diagonals extracted with a mask-multiply
   + strided reduce on the vector engine.
 - Per step: AllReduce(partial scores), AllGather(attn^T chunk),
   AllGather(h^T chunk) via ncfw collectives.
 - x@Wx + b is precomputed on the host (fp32 GEMM over bf16-rounded
   operands) and streamed per step.
Numerics: bf16 matmul operands, fp32 PSUM/state/softmax.
"""

import os
from contextlib import ExitStack

import numpy as np
import ml_dtypes

import concourse.bass as bass
import concourse.tile as tile
from concourse import bacc, mybir
from concourse.bass_utils import run_bass_kernel_spmd
from concourse.masks import make_identity

F32 = mybir.dt.float32
BF16 = mybir.dt.bfloat16
AF = mybir.ActivationFunctionType
OP = mybir.AluOpType

N, T, D, H = 128, 64, 512, 1024
L = 196
NCORES = 8
HS = H // NCORES          # 128 h-cols per core
CS = 4 * H // NCORES      # 512 gate cols per core
SCALE = 1.0 / float(np.sqrt(H))
G = 4                     # block-diag sample group size
NG = N // G               # 32 groups
BD = NG * 128             # block-diag lhsT total cols

TSTEPS = int(os.environ.get("KERNEL_TSTEPS", T))
RG = [list(range(NCORES))]


def _ap(t, dims, offset=0):
    """Custom strided view of a 2D tile: dims = [[step, count], ...] (free dims)."""
    a = t[:]
    return bass.AP(a.tensor, a.offset + offset, [a.ap[0]] + dims)


def build_nc(tsteps):
    nc = bacc.Bacc("TRN2", target_bir_lowering=False, debug=False,
                   num_devices=NCORES)
    d_a1 = nc.dram_tensor("a1", (128, N, L), BF16, kind="ExternalInput").ap()
    d_a2 = nc.dram_tensor("a2", (2, 128, N, HS), BF16, kind="ExternalInput").ap()
    d_w = nc.dram_tensor("wslab", (128, 16, CS), BF16, kind="ExternalInput").ap()
    d_xwb = nc.dram_tensor("xwb", (tsteps, N, CS), F32, kind="ExternalInput").ap()
    d_sm = nc.dram_tensor("smask", (128, 1024), F32, kind="ExternalInput").ap()
    d_rm = nc.dram_tensor("rmask", (128, G * HS), F32, kind="ExternalInput").ap()
    d_out = nc.dram_tensor("hout", (tsteps, N, HS), F32, kind="ExternalOutput").ap()

    with tile.TileContext(nc) as tc:
        with ExitStack() as ctx:
            _build(ctx, tc, tsteps, d_a1, d_a2, d_w, d_xwb, d_sm, d_rm, d_out)
    nc.compile()
    return nc


def _build(ctx, tc, tsteps, d_a1, d_a2, d_w, d_xwb, d_sm, d_rm, d_out):
    nc = tc.nc
    pp = ctx.enter_context(tc.tile_pool(name="persist", bufs=1))
    sb = ctx.enter_context(tc.tile_pool(name="work", bufs=2))
    ps_s = ctx.enter_context(tc.tile_pool(name="ps_s", bufs=1, space="PSUM"))
    ps_r = ctx.enter_context(tc.tile_pool(name="ps_r", bufs=1, space="PSUM"))
    ps_g = ctx.enter_context(tc.tile_pool(name="ps_g", bufs=2, space="PSUM"))
    ps_t = ctx.enter_context(tc.tile_pool(name="ps_t", bufs=2, space="PSUM"))
    ps_i = ctx.enter_context(tc.tile_pool(name="ps_i", bufs=1, space="PSUM"))
    dr = ctx.enter_context(tc.tile_pool(name="bounce", bufs=2, space="DRAM"))

    # ---- persistent tiles
    t_a1 = pp.tile([128, N * L], BF16)           # [h_loc | n, l]
    t_a2 = pp.tile([128, 2 * N * HS], BF16)      # [l_loc | lc, n, h_loc]
    t_w = pp.tile([128, 16 * CS], BF16)          # [hrow | chunk, col]
    t_sm = pp.tile([128, 1024], F32)
    t_rm = pp.tile([128, G * HS], F32)
    t_hbd = pp.tile([128, BD], BF16)             # block-diag h^T (scores lhsT)
    t_wbd0 = pp.tile([128, BD], BF16)            # block-diag w^T, l-chunk 0
    t_wbd1 = pp.tile([128, BD], BF16)            # block-diag w^T, l-chunk 1
    t_hTg = pp.tile([128, NCORES * 128], BF16)   # gathered h^T [hrow | c, n]
    t_aTg = pp.tile([128, NCORES * 128], BF16)   # gathered attn^T
    t_c = pp.tile([128, HS], F32)                # cell state (n-part, h-slice)
    t_id = pp.tile([128, 128], BF16)             # identity for PE transpose
    t_idf = pp.tile([128, 128], F32)

    # ---- loads
    nc.sync.dma_start(t_a1[:], d_a1.rearrange("p n l -> p (n l)"))
    nc.sync.dma_start(_ap(t_a2, [[N * HS, 2], [HS, N], [1, HS]]),
                      d_a2.rearrange("c p n h -> p c n h"))
    nc.sync.dma_start(t_w[:], d_w.rearrange("p c x -> p (c x)"))
    nc.sync.dma_start(t_sm[:], d_sm)
    nc.sync.dma_start(t_rm[:], d_rm)
    make_identity(nc, t_id[:])
    make_identity(nc, t_idf[:])
    nc.vector.memset(t_hbd[:], 0.0)
    nc.vector.memset(t_wbd0[:], 0.0)
    nc.vector.memset(t_wbd1[:], 0.0)

    # strided views for block-diag fills: src (p, 32, 4), dst diag blocks
    bd_dst = lambda t: _ap(t, [[132, NG], [1, G]])

    def fill_bd(dst_tile, src_ap128, rows=128):
        # src_ap128: (128, 128) AP (cols = samples); scatter cols into diag blocks
        src = bass.AP(src_ap128.tensor, src_ap128.offset,
                      [[src_ap128.ap[0][0], rows], [G, NG], [1, G]])
        d = bd_dst(dst_tile)
        dst = bass.AP(d.tensor, d.offset, [[d.ap[0][0], rows]] + d.ap[1:])
        nc.vector.tensor_copy(dst, src)

    # ---- h0 = mean_l A1  (raw sum first)
    h0raw = sb.tile([128, N], F32)
    nc.vector.tensor_reduce(h0raw[:], _ap(t_a1, [[L, N], [1, L]]),
                            axis=mybir.AxisListType.X, op=OP.add)
    # h^T (bf16, scaled) -> block-diag + AG bounce
    h0T = sb.tile([128, N], BF16)
    nc.scalar.activation(h0T[:], h0raw[:], AF.Copy, scale=1.0 / L)
    fill_bd(t_hbd, h0T[:])
    # c0 = transpose(h0raw)/L  (fp32)
    tp0 = ps_i.tile([128, 128], F32)
    nc.tensor.transpose(tp0[:], h0raw[:], t_idf[:])
    nc.scalar.activation(t_c[:], tp0[:], AF.Copy, scale=1.0 / L)

    def allgather_128(src_bf16_ap, dst_tile):
        bi = dr.tile([128, 128], BF16)
        bo = dr.tile([NCORES * 128, 128], BF16)
        nc.gpsimd.dma_start(bi[:], src_bf16_ap)
        nc.gpsimd.collective_compute(
            "AllGather", OP.bypass, ins=[bi.opt()], outs=[bo.opt()],
            replica_groups=RG)
        nc.sync.dma_start(_ap(dst_tile, [[128, NCORES], [1, 128]]),
                          bo[:].rearrange("(c p) n -> p c n", c=NCORES))

    allgather_128(h0T[:], t_hTg)

    for t in range(tsteps):
        # ===== scores partial: (128n, 196) over own h-slice
        s_ps = ps_s.tile([128, 1024], F32)
        for q in range(NG):
            lhs = t_hbd[:, q * 128:(q + 1) * 128]
            base = q * G * L
            nc.tensor.matmul(s_ps[:, 0:2 * L], lhs,
                             t_a1[:, base:base + 2 * L],
                             start=(q == 0), stop=(q == NG - 1))
            nc.tensor.matmul(s_ps[:, 512:512 + 2 * L], lhs,
                             t_a1[:, base + 2 * L:base + 4 * L],
                             start=(q == 0), stop=(q == NG - 1))
        stmp = sb.tile([128, 4 * L], F32)
        nc.vector.tensor_tensor(
            _ap(stmp, [[2 * L, 2], [L, 2], [1, L]]),
            _ap(s_ps, [[512, 2], [L, 2], [1, L]]),
            _ap(t_sm, [[512, 2], [L, 2], [1, L]]),
            op=OP.mult)
        sc = sb.tile([128, L], F32)
        nc.vector.tensor_reduce(sc[:], _ap(stmp, [[1, L], [L, 4]]),
                                axis=mybir.AxisListType.X, op=OP.add)
        # ===== AllReduce scores
        ari = dr.tile([128, L], F32)
        aro = dr.tile([128, L], F32)
        nc.gpsimd.dma_start(ari[:], sc[:])
        nc.gpsimd.collective_compute("AllReduce", OP.add, ins=[ari.opt()],
                                     outs=[aro.opt()], replica_groups=RG)
        ssum = sb.tile([128, L], F32)
        nc.sync.dma_start(ssum[:], aro[:])
        # ===== softmax (no max-subtraction; logits bounded ~|2.5|)
        ex = sb.tile([128, L], F32)
        esum = sb.tile([128, 1], F32)
        nc.scalar.activation(ex[:], ssum[:], AF.Exp, scale=SCALE,
                             accum_out=esum[:])
        rec = sb.tile([128, 1], F32)
        nc.vector.reciprocal(rec[:], esum[:])
        wbf = sb.tile([128, L], BF16)
        nc.vector.tensor_scalar_mul(wbf[:], ex[:], rec[:])
        # ===== w^T transposes -> block-diag lhsT
        for lc, (wbd, cols) in enumerate(((t_wbd0, 128), (t_wbd1, 68))):
            tpw = ps_t.tile([128, 128], BF16, tag="tpb")
            nc.tensor.transpose(tpw[0:cols, :], wbf[:, lc * 128:lc * 128 + cols],
                                t_id[:])
            fill_bd(wbd, tpw[:], rows=cols)
        # ===== readout: attn (128n, 128h_loc)
        r_ps = ps_r.tile([128, G * HS], F32)
        first = True
        for q in range(NG):
            for lc, wbd in enumerate((t_wbd0, t_wbd1)):
                base = lc * N * HS + q * G * HS
                nc.tensor.matmul(r_ps[:], wbd[:, q * 128:(q + 1) * 128],
                                 t_a2[:, base:base + G * HS],
                                 start=first, stop=(q == NG - 1 and lc == 1))
                first = False
        rtmp = sb.tile([128, G * HS], F32)
        nc.vector.tensor_tensor(rtmp[:], r_ps[:], t_rm[:], op=OP.mult)
        attnb = sb.tile([128, HS], BF16)
        attnf = sb.tile([128, HS], F32)
        nc.vector.tensor_reduce(attnf[:], _ap(rtmp, [[1, HS], [HS, G]]),
                                axis=mybir.AxisListType.X, op=OP.add)
        nc.scalar.copy(attnb[:], attnf[:])
        # attn^T chunk + AllGather
        tpa = ps_t.tile([128, 128], BF16, tag="tpb")
        nc.tensor.transpose(tpa[:], attnb[:], t_id[:])
        aT = sb.tile([128, 128], BF16)
        nc.vector.tensor_copy(aT[:], tpa[:])
        allgather_128(aT[:], t_aTg)
        # ===== gates: a = [h;attn] @ Wslab  (+ xwb streamed)
        g_ps = ps_g.tile([128, CS], F32)
        for c in range(NCORES):
            nc.tensor.matmul(g_ps[:], t_hTg[:, c * 128:(c + 1) * 128],
                             t_w[:, c * CS:(c + 1) * CS],
                             start=(c == 0), stop=False)
        for c in range(NCORES):
            nc.tensor.matmul(g_ps[:], t_aTg[:, c * 128:(c + 1) * 128],
                             t_w[:, (8 + c) * CS:(9 + c) * CS],
                             start=False, stop=(c == NCORES - 1))
        xw = sb.tile([128, CS], F32)
        nc.sync.dma_start(xw[:], d_xwb[t])
        asb = sb.tile([128, CS], F32)
        nc.vector.tensor_add(asb[:], g_ps[:], xw[:])
        # ===== pointwise LSTM
        sig = sb.tile([128, 384], F32)
        nc.scalar.activation(sig[:], asb[:, 0:384], AF.Sigmoid)
        tg = sb.tile([128, 128], F32)
        nc.scalar.activation(tg[:], asb[:, 384:512], AF.Tanh)
        c1 = sb.tile([128, 128], F32)
        nc.vector.tensor_mul(c1[:], sig[:, 128:256], t_c[:])   # f*c
        c2 = sb.tile([128, 128], F32)
        nc.vector.tensor_mul(c2[:], sig[:, 0:128], tg[:])      # i*g
        nc.vector.tensor_add(t_c[:], c1[:], c2[:])
        tch = sb.tile([128, 128], F32)
        nc.scalar.activation(tch[:], t_c[:], AF.Tanh)
        hf = sb.tile([128, 128], F32)
        nc.vector.tensor_mul(hf[:], sig[:, 256:384], tch[:])
        nc.sync.dma_start(d_out[t], hf[:])
        # ===== h^T for next step
        if t < tsteps - 1:
            hbf = sb.tile([128, 128], BF16)
            nc.scalar.copy(hbf[:], hf[:])
            tph = ps_t.tile([128, 128], BF16, tag="tpb")
            nc.tensor.transpose(tph[:], hbf[:], t_id[:])
            fill_bd(t_hbd, tph[:])
            hT = sb.tile([128, 128], BF16)
            nc.vector.tensor_copy(hT[:], tph[:])
            allgather_128(hT[:], t_hTg)


# ---------------------------------------------------------------------------
# host side
# ---------------------------------------------------------------------------
_NC_CACHE = {}


def _get_nc(tsteps):
    if tsteps not in _NC_CACHE:
        _NC_CACHE[tsteps] = build_nc(tsteps)
    return _NC_CACHE[tsteps]


def _bf(x):
    return x.astype(ml_dtypes.bfloat16)


def prepare_inputs(x, A, Wx, Wh, Wattn, b, tsteps):
    """Build per-core in_maps."""
    Af = A.reshape(N, H, L).astype(np.float32)
    xwb = (_bf(x.reshape(N * T, D)).astype(np.float32)
           @ _bf(Wx).astype(np.float32)).reshape(N, T, 4 * H) + b[None, None, :]

    smask = np.zeros((128, 1024), np.float32)
    for n in range(128):
        s = n % G
        smask[n, (s // 2) * 512 + (s % 2) * L:(s // 2) * 512 + (s % 2) * L + L] = 1.0
    rmask = np.zeros((128, G * HS), np.float32)
    for n in range(128):
        rmask[n, (n % G) * HS:(n % G + 1) * HS] = 1.0

    in_maps = []
    for k in range(NCORES):
        hk = slice(128 * k, 128 * (k + 1))
        cols = np.concatenate([np.arange(1024 * g + 128 * k,
                                         1024 * g + 128 * (k + 1))
                               for g in range(4)])
        a1 = _bf(Af[:, hk, :].transpose(1, 0, 2))              # (128h, N, L)
        a2t = Af[:, hk, :].transpose(2, 0, 1)                  # (L, N, 128h)
        a2 = np.zeros((2, 128, N, HS), ml_dtypes.bfloat16)
        a2[0] = _bf(a2t[0:128])
        a2[1, 0:68] = _bf(a2t[128:196])
        wsl = np.empty((128, 16, CS), ml_dtypes.bfloat16)
        for c in range(8):
            wsl[:, c, :] = _bf(Wh[128 * c:128 * (c + 1)][:, cols])
            wsl[:, 8 + c, :] = _bf(Wattn[128 * c:128 * (c + 1)][:, cols])
        in_maps.append({
            "a1": np.ascontiguousarray(a1),
            "a2": a2,
            "wslab": wsl,
            "xwb": np.ascontiguousarray(
                xwb[:, :tsteps, cols].transpose(1, 0, 2)).astype(np.float32),
            "smask": smask,
            "rmask": rmask,
        })
    return in_maps


def kernel(x, A, Wx, Wh, Wattn, b, _tsteps=None, _collect_res=None):
    tsteps = _tsteps or TSTEPS
    x = np.asarray(x, np.float32)
    A = np.asarray(A, np.float32)
    nc = _get_nc(tsteps)
    in_maps = prepare_inputs(x, A, np.asarray(Wx, np.float32),
                             np.asarray(Wh, np.float32),
                             np.asarray(Wattn, np.float32),
                             np.asarray(b, np.float32), tsteps)
    res = run_bass_kernel_spmd(nc, in_maps, core_ids=list(range(NCORES)))
    if _collect_res is not None:
        _collect_res.append((nc, in_maps, res))
    out = np.empty((N, tsteps, H), np.float32)
    for k in range(NCORES):
        out[:, :, 128 * k:128 * (k + 1)] = res.results[k]["hout"].transpose(1, 0, 2)
    if tsteps == T:
        return out
    full = np.zeros((N, T, H), np.float32)
    full[:, :tsteps] = out
    return full


# revision 15
# speedup vs baseline: 2.7262x; 2.7262x over previous
"""Trainium2 Bass kernel for nn_CaptioningRNN (attention LSTM over T=64 steps).

Strategy (8-way tensor-parallel over H / gate dim):
 - Core k owns h-slice hk = [128k, 128k+128) and gate columns
   cols_k = {1024*g + 128k .. +128 : g in i,f,o,g}.
 - A is sharded by h for scores (A1: [h_loc, n, l]) and by l for the
   attention readout (A2: [l, n, h_loc]); both bf16, resident in SBUF.
 - Per-sample attention einsums run on the PE as block-diagonal matmuls
   (zero-padded 32-col lhsT tiles, 4x concurrent via tile_position);
   diagonals extracted with mask-multiply + strided reduce on DVE.
 - Per step TWO collectives: AllGather(h^T chunk ++ scores partial) and
   AllGather(attn^T chunk); the scores AllReduce is folded into the
   first AG (partials summed on DVE after the gather).
 - x@Wx + b precomputed on host, streamed per step.
Numerics: bf16 matmul operands, fp32 PSUM/state/softmax.
"""

import os
from contextlib import ExitStack

import numpy as np
import ml_dtypes

import concourse.bass as bass
import concourse.tile as tile
from concourse import bacc, mybir
from concourse.bass_utils import run_bass_kernel_spmd
from concourse.masks import make_identity

F32 = mybir.dt.float32
BF16 = mybir.dt.bfloat16
AF = mybir.ActivationFunctionType
OP = mybir.AluOpType

N, T, D, H = 128, 64, 512, 1024
L = 196
NCORES = 8
HS = H // NCORES          # 128
CS = 4 * H // NCORES      # 512
SCALE = 1.0 / float(np.sqrt(H))
G = 4                     # readout block-diag sample group size
NG = N // G               # 32 readout groups
GS = 2                    # scores block-diag sample group size
NGS = N // GS             # 64 scores groups
CAT = 128 + L             # combined AG payload cols (h^T ++ scores partial)

TSTEPS = int(os.environ.get("KERNEL_TSTEPS", T))
PACK = bool(int(os.environ.get("KERNEL_PACK", "1")))
NOCOLL = False
RG = [list(range(NCORES))]


def _ap(t, dims, offset=0):
    a = t[:]
    return bass.AP(a.tensor, a.offset + offset, [a.ap[0]] + dims)


def build_nc(tsteps):
    nc = bacc.Bacc("TRN2", target_bir_lowering=False, debug=False,
                   num_devices=NCORES)
    d_a1 = nc.dram_tensor("a1", (128, N, L), BF16, kind="ExternalInput").ap()
    d_a2 = nc.dram_tensor("a2", (2, 128, N, HS), BF16, kind="ExternalInput").ap()
    d_w = nc.dram_tensor("wslab", (128, 16, CS), BF16, kind="ExternalInput").ap()
    d_xwb = nc.dram_tensor("xwb", (tsteps, N, CS), F32, kind="ExternalInput").ap()
    d_sm = nc.dram_tensor("smask", (128, GS * L), F32, kind="ExternalInput").ap()
    d_rm = nc.dram_tensor("rmask", (128, G * HS), F32, kind="ExternalInput").ap()
    d_out = nc.dram_tensor("hout", (tsteps, N, HS), F32, kind="ExternalOutput").ap()

    with tile.TileContext(nc) as tc:
        with ExitStack() as ctx:
            _build(ctx, tc, tsteps, d_a1, d_a2, d_w, d_xwb, d_sm, d_rm, d_out)
    nc.compile()
    return nc


def _build(ctx, tc, tsteps, d_a1, d_a2, d_w, d_xwb, d_sm, d_rm, d_out):
    nc = tc.nc
    pp = ctx.enter_context(tc.tile_pool(name="persist", bufs=1))
    sb = ctx.enter_context(tc.tile_pool(name="work", bufs=3))
    ps_s = ctx.enter_context(tc.tile_pool(name="ps_s", bufs=1, space="PSUM"))
    ps_r = ctx.enter_context(tc.tile_pool(name="ps_r", bufs=1, space="PSUM"))
    ps_g = ctx.enter_context(tc.tile_pool(name="ps_g", bufs=2, space="PSUM"))
    ps_t = ctx.enter_context(tc.tile_pool(name="ps_t", bufs=1, space="PSUM"))
    dr = ctx.enter_context(tc.tile_pool(name="bounce", bufs=4, space="DRAM"))

    # ---- persistent tiles
    t_a1 = pp.tile([128, N * L], BF16)           # [h_loc | n, l]
    t_a2 = pp.tile([128, 2 * N * HS], BF16)      # [l_loc | lc, n, h_loc]
    t_w = pp.tile([128, 16 * CS], BF16)          # [hrow | chunk, col]
    t_sm = pp.tile([128, GS * L], F32)
    t_rm = pp.tile([128, G * HS], F32)
    # block-diag lhsT tiles: 32-col per group, few live cols each
    t_hbd = pp.tile([128, NGS * 32], BF16)
    t_wbd0 = pp.tile([128, NG * 32], BF16)
    t_wbd1 = pp.tile([128, NG * 32], BF16)
    t_hTg = pp.tile([128, NCORES * 128], BF16)   # gathered h^T [hrow | c, n]
    t_scg = pp.tile([128, NCORES * L], BF16)     # gathered scores partials
    t_aTg = pp.tile([128, NCORES * 128], BF16)   # gathered attn^T
    t_c = pp.tile([128, HS], F32)
    t_id = pp.tile([128, 128], BF16)
    t_idf = pp.tile([128, 128], F32)

    # ---- loads
    nc.sync.dma_start(t_a1[:], d_a1.rearrange("p n l -> p (n l)"))
    nc.sync.dma_start(_ap(t_a2, [[N * HS, 2], [HS, N], [1, HS]]),
                      d_a2.rearrange("c p n h -> p c n h"))
    nc.sync.dma_start(t_w[:], d_w.rearrange("p c x -> p (c x)"))
    nc.sync.dma_start(t_sm[:], d_sm)
    nc.sync.dma_start(t_rm[:], d_rm)
    make_identity(nc, t_id[:])
    make_identity(nc, t_idf[:])
    nc.vector.memset(t_hbd[:], 0.0)
    if NOCOLL:
        nc.vector.memset(t_hTg[:], 0.0)
        nc.vector.memset(t_scg[:], 0.0)
        nc.vector.memset(t_aTg[:], 0.0)
    nc.vector.memset(t_wbd0[:], 0.0)
    nc.vector.memset(t_wbd1[:], 0.0)

    # block-diag fills. Sample s = 32a + G*b + j (block a, group-in-block b,
    # lane j) lands at dst col q*32 + (s - 32a) with q the global group id.
    def fill_bd_g(dst_tile, src_ap128, g, rows=128):
        npb = 32 // g                      # groups per 32-sample block
        src = bass.AP(src_ap128.tensor, src_ap128.offset,
                      [[src_ap128.ap[0][0], rows], [32, 4], [g, npb], [1, g]])
        d = _ap(dst_tile, [[npb * 32, 4], [32 + g, npb], [1, g]])
        dst = bass.AP(d.tensor, d.offset, [[d.ap[0][0], rows]] + d.ap[1:])
        nc.vector.tensor_copy(dst, src)

    def fill_bd(dst_tile, src_ap128, rows=128):      # readout (G=4)
        fill_bd_g(dst_tile, src_ap128, G, rows)

    def fill_bd_s(dst_tile, src_ap128, rows=128):    # scores (GS=2)
        fill_bd_g(dst_tile, src_ap128, GS, rows)

    # ---- scores partial: 64 MMs of 392 cols (4x col-packed)
    def scores_block(s_ps):
        for jc in range(4):
            tp = (0, 32 * jc) if PACK else None
            orng = slice(32 * jc, 32 * jc + 32) if PACK else slice(0, 128)
            for qq in range(16):
                q = 16 * jc + qq
                base = q * GS * L
                lhs = t_hbd[:, q * 32:(q + 1) * 32]
                nc.tensor.matmul(s_ps[orng, 0:GS * L], lhs,
                                 t_a1[:, base:base + GS * L],
                                 start=(qq == 0), stop=(qq == 15),
                                 tile_position=tp)

    def extract_scores(s_ps):
        stmp = sb.tile([128, GS * L], F32)
        nc.vector.tensor_tensor(stmp[:], s_ps[:, 0:GS * L], t_sm[:],
                                op=OP.mult)
        sc = sb.tile([128, L], F32)
        nc.vector.tensor_reduce(sc[:], _ap(stmp, [[1, L], [L, GS]]),
                                axis=mybir.AxisListType.X, op=OP.add)
        return sc

    def combined_ag(cat_tile):
        """AG of (128, CAT) bf16 = [h^T chunk | scores partial]."""
        if NOCOLL:
            nc.sync.dma_start(_ap(t_hTg, [[1, 128]]), cat_tile[:, 0:128])
            nc.sync.dma_start(_ap(t_scg, [[1, L]]), cat_tile[:, 128:CAT])
            return
        bi = dr.tile([128, CAT], BF16)
        bo = dr.tile([NCORES * 128, CAT], BF16)
        nc.sync.dma_start(bi[:], cat_tile[:])
        nc.gpsimd.collective_compute("AllGather", OP.bypass, ins=[bi.opt()],
                                     outs=[bo.opt()], replica_groups=RG)
        bov = bo[:].rearrange("(c p) x -> p c x", c=NCORES)
        nc.sync.dma_start(_ap(t_hTg, [[128, NCORES], [1, 128]]),
                          bov[:, :, 0:128])
        nc.sync.dma_start(_ap(t_scg, [[L, NCORES], [1, L]]),
                          bov[:, :, 128:CAT])

    # ---- init: h0 = mean_l A1
    h0raw = sb.tile([128, N], F32)
    nc.vector.tensor_reduce(h0raw[:], _ap(t_a1, [[L, N], [1, L]]),
                            axis=mybir.AxisListType.X, op=OP.add)
    h0T = sb.tile([128, N], BF16)
    nc.scalar.activation(h0T[:], h0raw[:], AF.Copy, scale=1.0 / L)
    fill_bd_s(t_hbd, h0T[:])
    tp0 = ps_t.tile([128, 128], F32, tag="tp0f")
    nc.tensor.transpose(tp0[:], h0raw[:], t_idf[:])
    nc.scalar.activation(t_c[:], tp0[:], AF.Copy, scale=1.0 / L)
    s_ps0 = ps_s.tile([128, 512], F32)
    scores_block(s_ps0)
    sc0 = extract_scores(s_ps0)
    cat0 = sb.tile([128, CAT], BF16, tag="cat")
    nc.vector.tensor_copy(cat0[:, 0:128], h0T[:])
    nc.vector.tensor_copy(cat0[:, 128:CAT], sc0[:])
    combined_ag(cat0)

    for t in range(tsteps):
        # ===== sum gathered scores partials, softmax
        ssum = sb.tile([128, L], F32)
        nc.vector.tensor_reduce(ssum[:], _ap(t_scg, [[1, L], [L, NCORES]]),
                                axis=mybir.AxisListType.X, op=OP.add)
        ex = sb.tile([128, L], F32)
        esum = sb.tile([128, 1], F32)
        nc.scalar.activation(ex[:], ssum[:], AF.Exp, scale=SCALE,
                             accum_out=esum[:])
        rec = sb.tile([128, 1], F32)
        nc.vector.reciprocal(rec[:], esum[:])
        wbf = sb.tile([128, L], BF16)
        nc.vector.tensor_scalar_mul(wbf[:], ex[:], rec[:])
        # ===== w^T transposes -> block-diag lhsT
        for lc, (wbd, cols) in enumerate(((t_wbd0, 128), (t_wbd1, 68))):
            tpw = ps_t.tile([128, 128], BF16, tag="tpb")
            nc.tensor.transpose(tpw[0:cols, :], wbf[:, lc * 128:lc * 128 + cols],
                                t_id[:])
            fill_bd(wbd, tpw[:], rows=cols)
        # ===== readout: attn (128n, 128h_loc), 64 MMs (4x packed)
        r_ps = ps_r.tile([128, G * HS], F32)
        for jc in range(4):
            tp = (0, 32 * jc) if PACK else None
            orng = slice(32 * jc, 32 * jc + 32) if PACK else slice(0, 128)
            for qq in range(8):
                q = 8 * jc + qq
                for lc, wbd in enumerate((t_wbd0, t_wbd1)):
                    base = lc * N * HS + q * G * HS
                    nc.tensor.matmul(r_ps[orng, :], wbd[:, q * 32:(q + 1) * 32],
                                     t_a2[:, base:base + G * HS],
                                     start=(qq == 0 and lc == 0),
                                     stop=(qq == 7 and lc == 1),
                                     tile_position=tp)
        rtmp = sb.tile([128, G * HS], F32)
        nc.vector.tensor_tensor(rtmp[:], r_ps[:], t_rm[:], op=OP.mult)
        attnf = sb.tile([128, HS], F32)
        nc.vector.tensor_reduce(attnf[:], _ap(rtmp, [[1, HS], [HS, G]]),
                                axis=mybir.AxisListType.X, op=OP.add)
        attnb = sb.tile([128, HS], BF16)
        nc.vector.tensor_copy(attnb[:], attnf[:])
        tpa = ps_t.tile([128, 128], BF16, tag="tpb")
        nc.tensor.transpose(tpa[:], attnb[:], t_id[:])
        aT = sb.tile([128, 128], BF16)
        nc.vector.tensor_copy(aT[:], tpa[:])
        # ===== AG2: attn^T chunk
        if NOCOLL:
            nc.sync.dma_start(_ap(t_aTg, [[1, 128]]), aT[:])
        else:
            bi = dr.tile([128, 128], BF16)
            bo = dr.tile([NCORES * 128, 128], BF16)
            nc.sync.dma_start(bi[:], aT[:])
            nc.gpsimd.collective_compute("AllGather", OP.bypass, ins=[bi.opt()],
                                         outs=[bo.opt()], replica_groups=RG)
            nc.sync.dma_start(_ap(t_aTg, [[128, NCORES], [1, 128]]),
                              bo[:].rearrange("(c p) n -> p c n", c=NCORES))
        # ===== gates
        g_ps = ps_g.tile([128, CS], F32)
        for c in range(NCORES):
            nc.tensor.matmul(g_ps[:], t_hTg[:, c * 128:(c + 1) * 128],
                             t_w[:, c * CS:(c + 1) * CS],
                             start=(c == 0), stop=False)
        for c in range(NCORES):
            nc.tensor.matmul(g_ps[:], t_aTg[:, c * 128:(c + 1) * 128],
                             t_w[:, (8 + c) * CS:(9 + c) * CS],
                             start=False, stop=(c == NCORES - 1))
        xw = sb.tile([128, CS], F32)
        nc.sync.dma_start(xw[:], d_xwb[t])
        asb = sb.tile([128, CS], F32)
        nc.vector.tensor_add(asb[:], g_ps[:], xw[:])
        # ===== pointwise LSTM
        sig = sb.tile([128, 384], F32)
        nc.scalar.activation(sig[:], asb[:, 0:384], AF.Sigmoid)
        tg = sb.tile([128, 128], F32)
        nc.scalar.activation(tg[:], asb[:, 384:512], AF.Tanh)
        c1 = sb.tile([128, 128], F32)
        nc.vector.tensor_mul(c1[:], sig[:, 128:256], t_c[:])
        c2 = sb.tile([128, 128], F32)
        nc.vector.tensor_mul(c2[:], sig[:, 0:128], tg[:])
        nc.vector.tensor_add(t_c[:], c1[:], c2[:])
        tch = sb.tile([128, 128], F32)
        nc.scalar.activation(tch[:], t_c[:], AF.Tanh)
        hf = sb.tile([128, 128], F32)
        nc.vector.tensor_mul(hf[:], sig[:, 256:384], tch[:])
        nc.sync.dma_start(d_out[t], hf[:])
        # ===== next-step h^T + scores partial + combined AG
        if t < tsteps - 1:
            hbf = sb.tile([128, 128], BF16)
            nc.vector.tensor_copy(hbf[:], hf[:])
            tph = ps_t.tile([128, 128], BF16, tag="tpb")
            nc.tensor.transpose(tph[:], hbf[:], t_id[:])
            fill_bd_s(t_hbd, tph[:])
            cat = sb.tile([128, CAT], BF16, tag="cat")
            nc.vector.tensor_copy(cat[:, 0:128], tph[:])
            s_ps = ps_s.tile([128, 512], F32)
            scores_block(s_ps)
            sc = extract_scores(s_ps)
            nc.vector.tensor_copy(cat[:, 128:CAT], sc[:])
            combined_ag(cat)


# ---------------------------------------------------------------------------
# host side
# ---------------------------------------------------------------------------
_NC_CACHE = {}


def _get_nc(tsteps):
    key = (tsteps, PACK)
    if key not in _NC_CACHE:
        _NC_CACHE[key] = build_nc(tsteps)
    return _NC_CACHE[key]


def _bf(x):
    return x.astype(ml_dtypes.bfloat16)


def prepare_inputs(x, A, Wx, Wh, Wattn, b, tsteps):
    Af = A.reshape(N, H, L).astype(np.float32)
    xwb = (_bf(x.reshape(N * T, D)).astype(np.float32)
           @ _bf(Wx).astype(np.float32)).reshape(N, T, 4 * H) + b[None, None, :]

    smask = np.zeros((128, GS * L), np.float32)
    for n in range(128):
        smask[n, (n % GS) * L:(n % GS) * L + L] = 1.0
    rmask = np.zeros((128, G * HS), np.float32)
    for n in range(128):
        rmask[n, (n % G) * HS:(n % G + 1) * HS] = 1.0

    in_maps = []
    for k in range(NCORES):
        hk = slice(128 * k, 128 * (k + 1))
        cols = np.concatenate([np.arange(1024 * g + 128 * k,
                                         1024 * g + 128 * (k + 1))
                               for g in range(4)])
        a1 = _bf(Af[:, hk, :].transpose(1, 0, 2))
        a2t = Af[:, hk, :].transpose(2, 0, 1)
        a2 = np.zeros((2, 128, N, HS), ml_dtypes.bfloat16)
        a2[0] = _bf(a2t[0:128])
        a2[1, 0:68] = _bf(a2t[128:196])
        wsl = np.empty((128, 16, CS), ml_dtypes.bfloat16)
        for c in range(8):
            wsl[:, c, :] = _bf(Wh[128 * c:128 * (c + 1)][:, cols])
            wsl[:, 8 + c, :] = _bf(Wattn[128 * c:128 * (c + 1)][:, cols])
        in_maps.append({
            "a1": np.ascontiguousarray(a1),
            "a2": a2,
            "wslab": wsl,
            "xwb": np.ascontiguousarray(
                xwb[:, :tsteps, cols].transpose(1, 0, 2)).astype(np.float32),
            "smask": smask,
            "rmask": rmask,
        })
    return in_maps


def kernel(x, A, Wx, Wh, Wattn, b, _tsteps=None):
    tsteps = _tsteps or TSTEPS
    x = np.asarray(x, np.float32)
    A = np.asarray(A, np.float32)
    nc = _get_nc(tsteps)
    in_maps = prepare_inputs(x, A, np.asarray(Wx, np.float32),
                             np.asarray(Wh, np.float32),
                             np.asarray(Wattn, np.float32),
                             np.asarray(b, np.float32), tsteps)
    res = run_bass_kernel_spmd(nc, in_maps, core_ids=list(range(NCORES)))
    out = np.empty((N, tsteps, H), np.float32)
    for k in range(NCORES):
        out[:, :, 128 * k:128 * (k + 1)] = res.results[k]["hout"].transpose(1, 0, 2)
    if tsteps == T:
        return out
    full = np.zeros((N, T, H), np.float32)
    full[:, :tsteps] = out
    return full
